# revision 22
# baseline (speedup 1.0000x reference)
"""BLT model TRN2 kernel — nn_BLTModel_13872744366807.

Strategy v3:
- Vocab collapse (v1): byte-axis path depends only on byte VALUE, so the
  [B,4096,*] byte axis collapses to a [B,256,*] vocab table; pooling is a
  host histogram matrix x emb; final output is a host gather.
- DP-2 x TP-4: cores 0-3 batch 0, cores 4-7 batch 1 (256 patch tokens).
- Attention reduction via AllGather of head-sharded attn outputs (bf16,
  ~9.5us vs ~18.5us AllReduce), Wo applied replicated per core.
- MLP reduction via 2 token-chunked fp16 AllReduces (~13us each),
  pipelined against the other chunk's w1/gelu/w2 and the next layer's
  qkv/scores lead-in (causal chunking: queries 0-127 need keys 0-127 only).
- LayerNorm: gpsimd partition_all_reduce produces partition-broadcast
  stats (no M=1 reduce / K=1 broadcast matmuls on PE); direct
  normalization z=(h-mu)*rsig with LN gains host-folded into weights;
  rsqrt = exp(-0.5*ln(var+eps)) so attention Exp and LN share one act
  table; gelu = Gelu_apprx_tanh (2e-4 end-to-end); dummy activations
  prefetch table swaps off the critical path.
- V projections computed in token-major layout (z as lhsT), so AV
  matmuls need no PE transposes; v/k/q biases host-folded where exact.
"""
import numpy as np
import ml_dtypes
import concourse.bacc as bacc
import concourse.bass as bass
import concourse.bass_isa as bass_isa
import concourse.mybir as mybir
from concourse import tile
from concourse.bass_utils import run_bass_kernel_spmd
from concourse.bass_interp import get_hw_module

F32 = mybir.dt.float32
BF16 = mybir.dt.bfloat16
FP16 = mybir.dt.float16
AF = mybir.ActivationFunctionType
ALU = mybir.AluOpType
ROP = bass_isa.ReduceOp
BF = ml_dtypes.bfloat16

L, B, S, P, H, V, NC = 4, 2, 4096, 256, 1024, 256, 8
G = 4                  # tensor-parallel group size
EPS = 1e-6
RG = [[0, 1, 2, 3], [4, 5, 6, 7]]

_CACHE = {}


def _trace(skip_kvn_ln):
    nc = bacc.Bacc("TRN2", target_bir_lowering=False, debug=False,
                   num_devices=NC)
    d = {}

    def inp(name, shape, dt=BF16):
        d[name] = nc.dram_tensor(name, shape, dt, kind="ExternalInput").ap()

    inp("wqkv", [L, 128, 4096])          # m-tiles [q0,k0,q1,k1] per kc
    inp("bqkv4", [L, 128, 4], F32)
    inp("wv", [L, 128, 2048])            # rhs layout [128, kc, 256]
    inp("wo", [L, 128, 8192])            # FULL Wo [128, kc, 1024]
    inp("bo8", [L, 128, 8], F32)
    inp("w1", [L, 128, 8192])
    inp("b1c", [L, 128, 8], F32)
    inp("w2", [L, 128, 8192])
    inp("b28", [L, 128, 8], F32)
    inp("wq", [128, 2048]); inp("wk", [128, 2048])
    inp("wvca", [128, 2048])             # rhs layout [128, kc, 256]
    inp("bq", [128, 2], F32); inp("bk", [128, 2], F32)
    inp("cawoT", [128, 2048])
    inp("headw", [128, 2048])
    inp("headb", [128, 2], F32)
    inp("embT", [128, 2048])
    inp("embS", [128, 2048])
    inp("cnt", [128, 512])
    inp("tri", [128, 128])
    inp("fng", [128, 8], F32); inp("fnb", [128, 8], F32)
    inp("cag", [128, 8], F32); inp("cab", [128, 8], F32)
    out_d = nc.dram_tensor("ltab", [128, 512], F32, kind="ExternalOutput").ap()

    with tile.TileContext(nc) as tc:
        with (
            tc.tile_pool(name="const", bufs=1) as cp,
            tc.tile_pool(name="sb", bufs=1) as sbp,
            tc.tile_pool(name="wts", bufs=1) as wp,
            tc.tile_pool(name="wts1", bufs=1) as wps,
            tc.tile_pool(name="tmp", bufs=2) as tp,
            tc.tile_pool(name="tps", bufs=2) as tps,
            tc.tile_pool(name="pp", bufs=1, space="PSUM") as pp,
            tc.tile_pool(name="pc", bufs=4, space="PSUM") as pc,
            tc.tile_pool(name="pa", bufs=3, space="PSUM") as pa,
            tc.tile_pool(name="dram", bufs=1, space="DRAM") as dp,
        ):
            # cc warm-up: tiny AllReduce to absorb launch skew
            wbin = dp.tile([128, 2], F32, tag="wrmi")
            wbout = dp.tile([128, 2], F32, tag="wrmo")
            nc.sync.dma_start(wbin[:], d["headb"][:])
            nc.gpsimd.collective_compute(
                "AllReduce", ALU.add, replica_groups=RG,
                ins=[wbin[:].opt()], outs=[wbout[:].opt()])

            # ---------------- constants ----------------
            def cload(name, shape, dt=BF16):
                t_ = cp.tile(shape, dt, tag=name)
                nc.sync.dma_start(t_[:], d[name][:])
                return t_

            tri_t = cload("tri", [128, 128])
            fng_t = cload("fng", [128, 8], F32); fnb_t = cload("fnb", [128, 8], F32)
            cag_t = cload("cag", [128, 8], F32); cab_t = cload("cab", [128, 8], F32)
            headb_t = cload("headb", [128, 2], F32)
            bq_t = cload("bq", [128, 2], F32); bk_t = cload("bk", [128, 2], F32)
            embS_t = cp.tile([128, 2, 1024], BF16, tag="embS")
            for q in range(2):
                nc.scalar.dma_start(embS_t[:, q, :], d["embS"][:].rearrange(
                    "p (vc x) -> p vc x", vc=2)[:, q, :])
            cnt_t = cp.tile([128, 2, 256], BF16, tag="cnt")
            nc.scalar.dma_start(cnt_t[:], d["cnt"][:].rearrange(
                "p (vc x) -> p vc x", vc=2))

            # ---------------- persistent activations ----------------
            h_t = sbp.tile([128, 8, 256], F32, tag="h")
            h16_t = sbp.tile([128, 8, 256], BF16, tag="h16")
            z_t = sbp.tile([128, 8, 256], BF16, tag="z")
            sq_t = sbp.tile([128, 8, 256], BF16, tag="sq")
            qk_t = sbp.tile([128, 2, 2, 256], BF16, tag="qk")
            qkh2_t = sbp.tile([64, 2, 2, 256], BF16, tag="qkh2")
            vtok_t = sbp.tile([128, 2, 256], BF16, tag="vtok")
            A_t = sbp.tile([128, 2, 256], BF16, tag="A")
            Af_t = sbp.tile([128, 8, 256], BF16, tag="Af")
            mo_t = sbp.tile([128, 8, 256], FP16, tag="mo")
            ari_t = sbp.tile([128, 8, 256], FP16, tag="ari")
            gu_t = sbp.tile([128, 8, 256], BF16, tag="gu")
            qn_t = gu_t  # tail_pre finishes with qn before gelu writes gu
            scrap_t = sbp.tile([1, 8], F32, tag="scrap")
            eps_t = sbp.tile([128, 1], F32, tag="eps")
            nc.vector.memset(eps_t[:], EPS)

            def prefetch_table(func):
                nc.scalar.activation(scrap_t[0:1, 0:1], tri_t[0:1, 0:1], func)

            # ---------------- LN stats + normalize helper ----------------
            def stats_z(src16, c, out_z, w, gain=None, bias=None):
                """src16: [128, 8, 256] bf16. Token slice cs=[c*w,(c+1)*w).
                Writes out_z[:, :, cs] = (x-mu)*rsig (*gain+bias per kc)."""
                cs = slice(c * w, c * w + w)
                nc.vector.tensor_tensor(out=sq_t[:, :, cs], in0=src16[:, :, cs],
                                        in1=src16[:, :, cs], op=ALU.mult)
                t4 = tp.tile([128, 4, 256], BF16, tag="tr4")
                t2 = tp.tile([128, 2, 256], BF16, tag="tr2")
                t1 = tps.tile([128, 512], BF16, tag="t1p")
                for (src_, dst_col) in ((src16, 0), (sq_t, 1)):
                    nc.vector.tensor_tensor(
                        out=t4[:, :, cs], in0=src_[:, 0:4, cs],
                        in1=src_[:, 4:8, cs], op=ALU.add)
                    nc.vector.tensor_tensor(
                        out=t2[:, :, :w], in0=t4[:, 0:2, cs],
                        in1=t4[:, 2:4, cs], op=ALU.add)
                    nc.vector.tensor_tensor(
                        out=t1[:, dst_col * w:(dst_col + 1) * w],
                        in0=t2[:, 0, :w], in1=t2[:, 1, :w], op=ALU.add)
                sb = tps.tile([128, 512], F32, tag="sb")
                nc.gpsimd.partition_all_reduce(
                    sb[:, 0:2 * w], t1[:, 0:2 * w],
                    channels=128, reduce_op=ROP.add)
                inv = 1.0 / float(H)
                mu = tps.tile([128, 256], F32, tag="mu")
                nc.vector.tensor_scalar_mul(mu[:, :w], sb[:, 0:w], inv)
                var = tps.tile([128, 256], F32, tag="var")
                # var = sumsq/H - mu*mu  (+EPS folded into Ln bias)
                nc.vector.scalar_tensor_tensor(
                    out=var[:, :w], in0=mu[:, :w], scalar=-1.0, in1=mu[:, :w],
                    op0=ALU.mult, op1=ALU.mult)
                nc.vector.scalar_tensor_tensor(
                    out=var[:, :w], in0=sb[:, w:2 * w], scalar=inv,
                    in1=var[:, :w], op0=ALU.mult, op1=ALU.add)
                nc.vector.tensor_scalar(out=var[:, :w], in0=var[:, :w],
                                        scalar1=EPS, scalar2=None, op0=ALU.add)
                # rsqrt: bit-trick seed + 2 Newton iterations (DVE only,
                # keeps Scalar act-table free for Exp/Gelu)
                rsig = tps.tile([128, 256], F32, tag="rsig")
                vi = var[:, :w].bitcast(mybir.dt.int32)
                yi = rsig[:, :w].bitcast(mybir.dt.int32)
                nc.vector.tensor_scalar(out=yi, in0=vi,
                                        scalar1=1, scalar2=None,
                                        op0=ALU.logical_shift_right)
                nc.vector.tensor_scalar(out=yi, in0=yi,
                                        scalar1=0x5f3759df, scalar2=-1,
                                        op0=ALU.subtract, op1=ALU.mult)
                nt = tps.tile([128, 256], F32, tag="nt")
                for _ in range(1):
                    nc.vector.tensor_tensor(out=nt[:, :w], in0=rsig[:, :w],
                                            in1=rsig[:, :w], op=ALU.mult)
                    nc.vector.tensor_tensor(out=nt[:, :w], in0=var[:, :w],
                                            in1=nt[:, :w], op=ALU.mult)
                    nc.vector.tensor_scalar(out=nt[:, :w], in0=nt[:, :w],
                                            scalar1=-0.5, scalar2=1.5,
                                            op0=ALU.mult, op1=ALU.add)
                    nc.vector.tensor_tensor(out=rsig[:, :w], in0=rsig[:, :w],
                                            in1=nt[:, :w], op=ALU.mult)
                r16 = tps.tile([128, 256], BF16, tag="r16")
                nc.vector.tensor_copy(r16[:, :w], rsig[:, :w])
                ms16 = tps.tile([128, 256], BF16, tag="ms16")
                nc.vector.tensor_tensor(out=ms16[:, :w], in0=mu[:, :w],
                                        in1=r16[:, :w], op=ALU.mult)
                z3 = out_z[:, :, cs]
                def bc8(t2d):
                    a = t2d[:, 0:w].rearrange("p (a b) -> p a b", a=1)
                    bb, _ = bass.broadcast_tensor_aps(a, z3)
                    return bb
                nc.vector.tensor_tensor(out=z3, in0=src16[:, :, cs],
                                        in1=bc8(r16), op=ALU.mult)
                nc.vector.tensor_tensor(out=z3, in0=z3,
                                        in1=bc8(ms16), op=ALU.subtract)
                if gain is not None:
                    for kc in range(8):
                        nc.vector.tensor_scalar(out=out_z[:, kc, cs],
                                                in0=out_z[:, kc, cs],
                                                scalar1=gain[:, kc:kc + 1],
                                                scalar2=bias[:, kc:kc + 1],
                                                op0=ALU.mult, op1=ALU.add)

            # ---------------- batch-independent tail precompute -------------
            embT_t = sbp.tile([128, 8, 256], BF16, tag="embT")
            headw_t = sbp.tile([128, 8, 256], BF16, tag="headw")
            wqca_t = sbp.tile([128, 8, 256], BF16, tag="wqca")
            cawoT_t = sbp.tile([128, 8, 256], BF16, tag="cawoT")

            def tail_loads():
                for (t_, nm) in ((embT_t, "embT"), (headw_t, "headw"),
                                 (wqca_t, "wq"), (cawoT_t, "cawoT")):
                    nc.scalar.dma_start(t_[:], d[nm][:].rearrange(
                        "p (kc x) -> p kc x", kc=8))

            qT_t = sbp.tile([128, 2, 256], BF16, tag="qT")
            w2c_t = sbp.tile([128, 2, 256], BF16, tag="w2c")
            et_t = sbp.tile([128, 2, 256], F32, tag="et")

            def tail_pre():
                if skip_kvn_ln:
                    stats_z(embT_t, 0, qn_t, 256)
                else:
                    stats_z(embT_t, 0, qn_t, 256, gain=cag_t, bias=cab_t)
                for h2 in range(2):
                    ps = pp.tile([128, 256], F32, tag="mm")
                    for kc in range(8):
                        nc.tensor.matmul(ps[:],
                                         wqca_t[:, kc, h2 * 128:(h2 + 1) * 128],
                                         qn_t[:, kc, :],
                                         start=(kc == 0), stop=(kc == 7))
                    nc.vector.tensor_scalar(out=qT_t[:, h2, :], in0=ps[:],
                                            scalar1=bq_t[:, h2:h2 + 1],
                                            scalar2=None, op0=ALU.add)
                for lt in range(2):
                    ps_e = pp.tile([128, 256], F32, tag="mm")
                    for kc in range(8):
                        nc.tensor.matmul(
                            ps_e[:], headw_t[:, kc, lt * 128:(lt + 1) * 128],
                            embT_t[:, kc, :],
                            start=(kc == 0), stop=(kc == 7))
                    nc.vector.tensor_copy(et_t[:, lt, :], ps_e[:])
                for h2 in range(2):
                    ps = pp.tile([128, 256], F32, tag="mm")
                    for kc in range(8):
                        nc.tensor.matmul(
                            ps[:], cawoT_t[:, kc, h2 * 128:(h2 + 1) * 128],
                            headw_t[:, kc, :],
                            start=(kc == 0), stop=(kc == 7))
                    nc.vector.tensor_copy(w2c_t[:, h2, :], ps[:])

            # ---------------- patch pooling: h = patchesT ----------------
            for ti in range(8):
                ps = pp.tile([128, 256], F32, tag="mm")
                for vc in range(2):
                    nc.tensor.matmul(ps[:],
                                     embS_t[:, vc, ti * 128:(ti + 1) * 128],
                                     cnt_t[:, vc, :],
                                     start=(vc == 0), stop=(vc == 1))
                nc.vector.tensor_copy(h_t[:, ti, :], ps[:])
                nc.scalar.activation(h16_t[:, ti, :], ps[:], AF.Copy)

            # initial ln1(L0) for both chunks
            stats_z(h16_t, 0, z_t, 128)
            stats_z(h16_t, 1, z_t, 128)

            # ---------------- transformer layers ----------------
            wqkv_t = wp.tile([128, 8, 512], BF16, tag="wqkv")
            wv_t = wp.tile([128, 8, 256], BF16, tag="wv")
            bqkv4_t = wp.tile([128, 4], F32, tag="bqkv4")
            wo_t = wps.tile([128, 8, 1024], BF16, tag="wo")
            bo8_t = wp.tile([128, 8], F32, tag="bo8")
            w1_t = wps.tile([128, 8, 1024], BF16, tag="w1")
            w2_t = wps.tile([128, 8, 1024], BF16, tag="w2")
            b1c_t = wp.tile([128, 8], F32, tag="b1c")
            b28_t = wp.tile([128, 8], F32, tag="b28")

            def load_attn_w(l):
                for q in range(2):
                    nc.scalar.dma_start(
                        wqkv_t[:, q * 4:(q + 1) * 4, :],
                        d["wqkv"][l].rearrange("p (kc x) -> p kc x", kc=8)
                        [:, q * 4:(q + 1) * 4, :])
                nc.scalar.dma_start(wv_t[:], d["wv"][l].rearrange(
                    "p (kc x) -> p kc x", kc=8))
                nc.scalar.dma_start(bqkv4_t[:], d["bqkv4"][l])

            def load_mlp_w(l):
                for (t_, nm, spl) in ((wo_t, "wo", 4), (w1_t, "w1", 4),
                                      (w2_t, "w2", 4)):
                    for q in range(spl):
                        nc.scalar.dma_start(
                            t_[:, q * 2:(q + 1) * 2, :],
                            d[nm][l].rearrange("p (kc x) -> p kc x", kc=8)
                            [:, q * 2:(q + 1) * 2, :])
                nc.scalar.dma_start(bo8_t[:], d["bo8"][l])
                nc.scalar.dma_start(b1c_t[:], d["b1c"][l])
                nc.scalar.dma_start(b28_t[:], d["b28"][l])

            for l in range(4):
                load_attn_w(l)
                load_mlp_w(l)
                if l == 0:
                    tail_loads()

                # ---- qkv projections (both chunks) + v_tok ----
                for c in range(2):
                    cs = slice(c * 128, c * 128 + 128)
                    for m in range(4):
                        p_, j = m // 2, m % 2
                        ps_f = pc.tile([128, 256], F32, tag="mm")
                        ps = ps_f[:, 0:128]
                        for kc in range(8):
                            nc.tensor.matmul(
                                ps[:], wqkv_t[:, kc, m * 128:(m + 1) * 128],
                                z_t[:, kc, cs],
                                start=(kc == 0), stop=(kc == 7))
                        nc.vector.tensor_scalar(
                            out=qk_t[:, p_, j, cs], in0=ps[:],
                            scalar1=bqkv4_t[:, m:m + 1],
                            scalar2=None, op0=ALU.add)
                        nc.sync.dma_start(qkh2_t[:, p_, j, cs],
                                          qk_t[64:128, p_, j, cs])
                    ps_v = pc.tile([128, 256], F32, tag="mm")
                    for kc in range(8):
                        nc.tensor.matmul(ps_v[:], z_t[:, kc, cs],
                                         wv_t[:, kc, :],
                                         start=(kc == 0), stop=(kc == 7))
                    nc.vector.tensor_copy(vtok_t[:, c, :], ps_v[:])

                # ---- attention: chunk c queries attend key-blocks <= c ----
                ag_outs = []
                em_c0 = tp.tile([128, 4, 128], BF16, tag="emc0")
                em_c1 = tp.tile([128, 2, 4, 128], BF16, tag="emc1")
                ems_c1 = tp.tile([128, 4, 128], BF16, tag="emsc1")
                for c in range(2):
                    cs = slice(c * 128, c * 128 + 128)
                    for hh in range(2):
                        for p_ in range(2):
                            hidx = p_ * 2 + hh
                            src = qk_t if hh == 0 else qkh2_t
                            qT = src[0:64, p_, 0, cs]
                            for kt in range(c + 1):
                                ks = slice(kt * 128, kt * 128 + 128)
                                kT = src[0:64, p_, 1, ks]
                                ps_sf = pa.tile([128, 256], F32, tag="att")
                                ps_s = ps_sf[:, 0:128]
                                nc.tensor.matmul(ps_s[:], kT, qT,
                                                 start=True, stop=True)
                                if c == kt:
                                    ex = tp.tile([128, 128], BF16, tag="ex")
                                    nc.scalar.activation(ex[:], ps_s[:],
                                                         AF.Exp, scale=0.125)
                                    dst = (em_c0[:, hidx, :] if c == 0 else
                                           em_c1[:, kt, hidx, :])
                                    nc.vector.tensor_tensor(
                                        out=dst, in0=ex[:], in1=tri_t[:],
                                        op=ALU.mult)
                                else:
                                    nc.scalar.activation(
                                        em_c1[:, kt, hidx, :], ps_s[:],
                                        AF.Exp, scale=0.125)
                    # denominators (partition reduce, output broadcast)
                    den = tps.tile([128, 4, 128], F32, tag="den")
                    if c == 0:
                        nc.gpsimd.partition_all_reduce(
                            den[:].rearrange("p a b -> p (a b)"),
                            em_c0[:].rearrange("p a b -> p (a b)"),
                            channels=128, reduce_op=ROP.add)
                    else:
                        nc.vector.tensor_tensor(out=ems_c1[:],
                                                in0=em_c1[:, 0], in1=em_c1[:, 1],
                                                op=ALU.add)
                        nc.gpsimd.partition_all_reduce(
                            den[:].rearrange("p a b -> p (a b)"),
                            ems_c1[:].rearrange("p a b -> p (a b)"),
                            channels=128, reduce_op=ROP.add)
                    rec = tps.tile([128, 4, 128], F32, tag="rec")
                    nc.vector.reciprocal_approx_fast(
                        out=rec[:].rearrange("p a b -> p (a b)"),
                        in_=den[:].rearrange("p a b -> p (a b)"))
                    # AV + scale
                    for p_ in range(2):
                        ps_of = pa.tile([128, 256], F32, tag="att")
                        ps_o = ps_of[:, 0:128]
                        for hh in range(2):
                            hidx = p_ * 2 + hh
                            for kt in range(c + 1):
                                ks = slice(kt * 128, kt * 128 + 128)
                                em_ap = (em_c0[:, hidx, :] if c == 0 else
                                         em_c1[:, kt, hidx, :])
                                nc.tensor.matmul(
                                    ps_o[hh * 64:(hh + 1) * 64, :],
                                    vtok_t[:, kt, hidx * 64:(hidx + 1) * 64],
                                    em_ap, start=(kt == 0), stop=(kt == c))
                        for hh in range(2):
                            hidx = p_ * 2 + hh
                            nc.vector.tensor_tensor(
                                out=A_t[hh * 64:(hh + 1) * 64, p_, cs],
                                in0=ps_o[hh * 64:(hh + 1) * 64, :],
                                in1=rec[hh * 64:(hh + 1) * 64, hidx, :],
                                op=ALU.mult)
                    # stage + AllGather this chunk's attn output
                    ag_in = dp.tile([128, 2, 128], BF16, tag=f"agi{l}{c}")
                    ag_out = dp.tile([4, 128, 2, 128], BF16, tag=f"ago{l}{c}")
                    nc.sync.dma_start(ag_in[:], A_t[:, :, cs])
                    nc.gpsimd.collective_compute(
                        "AllGather", ALU.bypass, replica_groups=RG,
                        ins=[ag_in[:].opt()], outs=[ag_out[:].opt()])
                    ag_outs.append(ag_out)
                    if l == 0 and c == 0:
                        tail_pre()
                # fetches after both stages: sync queue stays monotone
                for c in range(2):
                    cs = slice(c * 128, c * 128 + 128)
                    for r_ in range(4):
                        nc.sync.dma_start(Af_t[:, 2 * r_:2 * r_ + 2, cs],
                                          ag_outs[c][r_])

                # ---- wo (full) + resid + ln2 + mlp; wo(c1) split so its
                # first half fills the PE gap while stats(c0) runs ----
                def wo_block(c, m0, m1):
                    cs = slice(c * 128, c * 128 + 128)
                    for m in range(m0, m1):
                        ps_f = pc.tile([128, 256], F32, tag="mm")
                        ps = ps_f[:, 0:128]
                        for kc in range(8):
                            nc.tensor.matmul(
                                ps[:], wo_t[:, kc, m * 128:(m + 1) * 128],
                                Af_t[:, kc, cs],
                                start=(kc == 0), stop=(kc == 7))
                        nc.vector.scalar_tensor_tensor(
                            out=h_t[:, m, cs], in0=ps[:],
                            scalar=bo8_t[:, m:m + 1], in1=h_t[:, m, cs],
                            op0=ALU.add, op1=ALU.add)
                        nc.scalar.activation(h16_t[:, m, cs], h_t[:, m, cs],
                                             AF.Copy)

                def mlp_block(c, ltag):
                    cs = slice(c * 128, c * 128 + 128)
                    for m in range(8):
                        ps_f = pc.tile([128, 256], F32, tag="mm")
                        ps = ps_f[:, 0:128]
                        for kc in range(8):
                            nc.tensor.matmul(
                                ps[:], w1_t[:, kc, m * 128:(m + 1) * 128],
                                z_t[:, kc, cs],
                                start=(kc == 0), stop=(kc == 7))
                        nc.scalar.activation(gu_t[:, m, cs], ps[:],
                                             AF.Gelu_apprx_tanh,
                                             bias=b1c_t[:, m:m + 1])
                    for m in range(8):
                        ps_f = pc.tile([128, 256], F32, tag="mm")
                        ps = ps_f[:, 0:128]
                        for kc in range(8):
                            nc.tensor.matmul(
                                ps[:], w2_t[:, kc, m * 128:(m + 1) * 128],
                                gu_t[:, kc, cs],
                                start=(kc == 0), stop=(kc == 7))
                        nc.vector.tensor_scalar(out=mo_t[:, m, cs], in0=ps[:],
                                                scalar1=b28_t[:, m:m + 1],
                                                scalar2=None, op0=ALU.add)
                    ar_in = dp.tile([128, 1024], FP16, tag=f"arin{ltag}")
                    ar_out = dp.tile([128, 1024], FP16, tag=f"arou{ltag}")
                    nc.sync.dma_start(
                        ar_in[:].rearrange("p (a b) -> p a b", a=8),
                        mo_t[:, :, cs])
                    nc.gpsimd.collective_compute(
                        "AllReduce", ALU.add, replica_groups=RG,
                        ins=[ar_in[:].opt()], outs=[ar_out[:].opt()])
                    return ar_out

                arb = [None, None]
                wo_block(0, 0, 8)
                stats_z(h16_t, 0, z_t, 128)
                wo_block(1, 0, 4)
                prefetch_table(AF.Gelu_apprx_tanh)
                arb[0] = mlp_block(0, f"{l}0")
                wo_block(1, 4, 8)
                stats_z(h16_t, 1, z_t, 128)
                arb[1] = mlp_block(1, f"{l}1")
                prefetch_table(AF.Exp)

                # ---- mlp resid + next ln1 (or final norm), per chunk ----
                for c in range(2):
                    cs = slice(c * 128, c * 128 + 128)
                    nc.sync.dma_start(
                        ari_t[:, :, cs],
                        arb[c][:].rearrange("p (a b) -> p a b", a=8))
                    for m in range(8):
                        nc.vector.tensor_tensor(out=h_t[:, m, cs],
                                                in0=h_t[:, m, cs],
                                                in1=ari_t[:, m, cs], op=ALU.add)
                        nc.scalar.activation(h16_t[:, m, cs], h_t[:, m, cs],
                                             AF.Copy)
                    if l < 3 or skip_kvn_ln:
                        stats_z(h16_t, c, z_t, 128)
                    else:
                        stats_z(h16_t, c, z_t, 128, gain=fng_t, bias=fnb_t)

            # ---------------- tail: final norm / CA / logits ----------------
            # z_t now holds pf = ln(h)*fng+fnb (or plain z if skip).
            if not skip_kvn_ln:
                # kvn = ln(pf)*cag+cab ; pf currently in z_t -> copy to h16
                for ti in range(8):
                    nc.vector.tensor_copy(h16_t[:, ti, :], z_t[:, ti, :])
                stats_z(h16_t, 0, z_t, 256, gain=cag_t, bias=cab_t)
            kvn_t = z_t

            wkca_t = cp.tile([128, 8, 256], BF16, tag="wkca")
            nc.scalar.dma_start(wkca_t[:], d["wk"][:].rearrange(
                "p (kc x) -> p kc x", kc=8))
            wvca_t = cp.tile([128, 8, 256], BF16, tag="wvca")
            nc.scalar.dma_start(wvca_t[:], d["wvca"][:].rearrange(
                "p (kc x) -> p kc x", kc=8))

            kT_t = sbp.tile([128, 2, 256], BF16, tag="kT")
            vtca_t = sbp.tile([128, 2, 256], BF16, tag="vtca")
            for h2 in range(2):
                ps = pp.tile([128, 256], F32, tag="mm")
                for kc in range(8):
                    nc.tensor.matmul(ps[:],
                                     wkca_t[:, kc, h2 * 128:(h2 + 1) * 128],
                                     kvn_t[:, kc, :],
                                     start=(kc == 0), stop=(kc == 7))
                nc.vector.tensor_scalar(out=kT_t[:, h2, :], in0=ps[:],
                                        scalar1=bk_t[:, h2:h2 + 1],
                                        scalar2=None, op0=ALU.add)
            for tb in range(2):
                ps_v = pp.tile([128, 256], F32, tag="mm")
                for kc in range(8):
                    nc.tensor.matmul(ps_v[:],
                                     kvn_t[:, kc, tb * 128:(tb + 1) * 128],
                                     wvca_t[:, kc, :],
                                     start=(kc == 0), stop=(kc == 7))
                nc.vector.tensor_copy(vtca_t[:, tb, :], ps_v[:])

            # ---------------- CA attention (2 heads, dh=128) ----------------
            O_t = sbp.tile([128, 2, 256], BF16, tag="O")
            emca = tp.tile([128, 2, 2, 256], BF16, tag="emca")
            for h2 in range(2):
                for kt in range(2):
                    ps_s = pa.tile([128, 256], F32, tag="att")
                    nc.tensor.matmul(
                        ps_s[:], kT_t[:, h2, kt * 128:(kt + 1) * 128],
                        qT_t[:, h2, :], start=True, stop=True)
                    nc.scalar.activation(emca[:, h2, kt, :], ps_s[:], AF.Exp,
                                         scale=float(1.0 / np.sqrt(128.0)))
            emsca = tp.tile([128, 2, 256], BF16, tag="emsca")
            nc.vector.tensor_tensor(out=emsca[:], in0=emca[:, :, 0, :],
                                    in1=emca[:, :, 1, :], op=ALU.add)
            denca = tps.tile([128, 2, 256], F32, tag="denca")
            nc.gpsimd.partition_all_reduce(
                denca[:].rearrange("p a b -> p (a b)"),
                emsca[:].rearrange("p a b -> p (a b)"),
                channels=128, reduce_op=ROP.add)
            recca = tps.tile([128, 2, 256], F32, tag="recca")
            nc.vector.reciprocal_approx_fast(
                out=recca[:].rearrange("p a b -> p (a b)"),
                in_=denca[:].rearrange("p a b -> p (a b)"))
            for h2 in range(2):
                ps_o = pa.tile([128, 256], F32, tag="att")
                for kt in range(2):
                    nc.tensor.matmul(
                        ps_o[:], vtca_t[:, kt, h2 * 128:(h2 + 1) * 128],
                        emca[:, h2, kt, :], start=(kt == 0), stop=(kt == 1))
                nc.vector.tensor_tensor(out=O_t[:, h2, :], in0=ps_o[:],
                                        in1=recca[:, h2, :], op=ALU.mult)

            # ---------------- logits partials + AR ----------------
            lp_t = sbp.tile([128, 2, 256], FP16, tag="lp")
            for lt in range(2):
                ps = pp.tile([128, 256], F32, tag="mm")
                for h2 in range(2):
                    nc.tensor.matmul(ps[:],
                                     w2c_t[:, h2, lt * 128:(lt + 1) * 128],
                                     O_t[:, h2, :],
                                     start=(h2 == 0), stop=(h2 == 1))
                nc.vector.tensor_copy(lp_t[:, lt, :], ps[:])
            lbin = dp.tile([128, 512], FP16, tag="lci")
            lbout = dp.tile([128, 512], FP16, tag="lco")
            nc.sync.dma_start(lbin[:], lp_t[:])
            nc.gpsimd.collective_compute(
                "AllReduce", ALU.add, replica_groups=RG,
                ins=[lbin[:].opt()], outs=[lbout[:].opt()])
            lar_t = sbp.tile([128, 2, 256], FP16, tag="lar")
            nc.sync.dma_start(lar_t[:], lbout[:])

            out_t = sbp.tile([128, 2, 256], F32, tag="outt")
            for lt in range(2):
                tb = tp.tile([128, 256], F32, tag="tb")
                nc.vector.tensor_scalar(out=tb[:], in0=lar_t[:, lt, :],
                                        scalar1=headb_t[:, lt:lt + 1],
                                        scalar2=None, op0=ALU.add)
                nc.vector.tensor_tensor(out=out_t[:, lt, :],
                                        in0=tb[:], in1=et_t[:, lt, :],
                                        op=ALU.add)
            nc.sync.dma_start(out_d[:], out_t[:])

    nc.compile()
    nc.m = get_hw_module(nc.m)
    return nc


# --------------------------------------------------------------------------
# host side
# --------------------------------------------------------------------------
def _shuf(M):
    """[K, X] -> [128, (K//128)*X] laid out as [p, kc, x]."""
    K, X = M.shape
    return np.ascontiguousarray(
        M.reshape(K // 128, 128, X).transpose(1, 0, 2).reshape(128, -1))


def _bf(M):
    return np.ascontiguousarray(M).astype(BF)


def _prep(inputs):
    f = lambda k: np.asarray(inputs[k], np.float32)
    byte_seq = np.asarray(inputs["byte_seq"])
    bd = np.asarray(inputs["patch_boundaries"])
    emb = f("emb")

    # patch histogram matrix
    pos = np.arange(S)
    pid = np.stack([np.searchsorted(bd[b], pos, side="right") for b in range(B)])
    pid = np.clip(pid, 0, P - 1)
    Cn = np.zeros((B, P, V), np.float32)
    for b in range(B):
        np.add.at(Cn[b], (pid[b], byte_seq[b]), 1.0)
    cnts = Cn.sum(-1)
    Cn /= np.maximum(cnts, 1.0)[..., None]

    g1, b1a = f("g_ln1_g"), f("g_ln1_b")
    g2, b2a = f("g_ln2_g"), f("g_ln2_b")
    Wqkv, bqkv = f("g_wqkv"), f("g_bqkv")
    Wo, bo = f("g_wo"), f("g_bo")
    W1, b1 = f("g_w1"), f("g_b1")
    W2, b2 = f("g_w2"), f("g_b2")

    Wq_f = g1[:, :, None] * Wqkv                       # [L, H, 3H]
    biasq = np.einsum("lh,lho->lo", b1a, Wqkv) + bqkv  # [L, 3H]
    W1_f = g2[:, :, None] * W1
    bias1 = np.einsum("lh,lho->lo", b2a, W1) + b1

    # v bias folds into wo bias: attn_out = A_nobias + bv  (softmax sums to 1)
    bv_full = biasq[:, 2 * H:]                          # [L, H]
    bo_full = bo + np.einsum("lh,lho->lo", bv_full, Wo)  # [L, H]

    ca_wqkv, ca_bqkv = f("ca_wqkv"), f("ca_bqkv")
    ca_wo, ca_bo = f("ca_wo"), f("ca_bo")
    head_w, head_b = f("head_w"), f("head_b")
    headb_full = (head_b + ca_bo @ head_w
                  + (ca_bqkv[2 * H:] @ ca_wo) @ head_w)  # [256]

    tri = (np.arange(128)[:, None] <= np.arange(128)[None, :]).astype(
        np.float32)

    shared = {
        "headw": _bf(_shuf(head_w)),
        "headb": np.ascontiguousarray(headb_full.reshape(2, 128).T),
        "embT": _bf(_shuf(np.ascontiguousarray(emb.T))),
        "embS": _bf(_shuf(emb)),
        "tri": _bf(tri),
        "fng": np.ascontiguousarray(f("fn_g").reshape(8, 128).T),
        "fnb": np.ascontiguousarray(f("fn_b").reshape(8, 128).T),
        "cag": np.ascontiguousarray(f("ca_ln_g").reshape(8, 128).T),
        "cab": np.ascontiguousarray(f("ca_ln_b").reshape(8, 128).T),
        "wo": _bf(np.stack([_shuf(Wo[l]) for l in range(L)])),
        "bo8": np.ascontiguousarray(
            bo_full.reshape(L, 8, 128).transpose(0, 2, 1)),
    }

    in_maps = []
    for c in range(NC):
        g, r = c // G, c % G
        m = dict(shared)
        m["cnt"] = _bf(_shuf(np.ascontiguousarray(Cn[g].T)))

        # q/k m-tiles: [q_p0, k_p0, q_p1, k_p1]
        qk_cols = np.concatenate([
            j * H + 256 * r + 128 * p + np.arange(128)
            for p in range(2) for j in range(2)])
        m["wqkv"] = np.stack([_bf(_shuf(Wq_f[l][:, qk_cols]))
                              for l in range(L)])
        m["bqkv4"] = np.ascontiguousarray(
            biasq[:, qk_cols].reshape(L, 4, 128).transpose(0, 2, 1))
        # v as rhs [128, kc, 256]: cols 2H + 256r .. +256
        vsl = slice(2 * H + 256 * r, 2 * H + 256 * (r + 1))
        m["wv"] = np.stack([_bf(_shuf(Wq_f[l][:, vsl])) for l in range(L)])

        csl = slice(1024 * r, 1024 * (r + 1))
        m["w1"] = np.stack([_bf(_shuf(W1_f[l][:, csl])) for l in range(L)])
        m["b1c"] = np.ascontiguousarray(
            bias1[:, csl].reshape(L, 8, 128).transpose(0, 2, 1))
        m["w2"] = np.stack([_bf(_shuf(W2[l][csl, :])) for l in range(L)])
        m["b28"] = np.ascontiguousarray(
            b2.reshape(L, 8, 128).transpose(0, 2, 1) / G)

        hsl = slice(256 * r, 256 * (r + 1))
        m["wq"] = _bf(_shuf(ca_wqkv[:, hsl]))
        m["wk"] = _bf(_shuf(ca_wqkv[:, np.arange(256) + H + 256 * r]))
        m["wvca"] = _bf(_shuf(ca_wqkv[:, np.arange(256) + 2 * H + 256 * r]))
        m["bq"] = np.ascontiguousarray(ca_bqkv[hsl].reshape(2, 128).T)
        m["bk"] = np.ascontiguousarray(
            ca_bqkv[H + 256 * r:H + 256 * (r + 1)].reshape(2, 128).T)
        m["cawoT"] = _bf(np.concatenate([
            _shuf(np.ascontiguousarray(
                ca_wo[256 * r + 128 * h2:256 * r + 128 * (h2 + 1), :].T))
            .reshape(128, 8, 128) for h2 in range(2)], axis=2)
            .reshape(128, -1))
        in_maps.append(m)
    return in_maps, byte_seq


def run_device(inputs, trace=False):
    skip = (np.allclose(np.asarray(inputs["fn_g"]), 1.0)
            and np.allclose(np.asarray(inputs["fn_b"]), 0.0)
            and np.allclose(np.asarray(inputs["ca_ln_g"]), 1.0)
            and np.allclose(np.asarray(inputs["ca_ln_b"]), 0.0))
    key = ("nc", skip)
    if key not in _CACHE:
        _CACHE[key] = _trace(skip)
    nc = _CACHE[key]
    in_maps, byte_seq = _prep(inputs)
    res = run_bass_kernel_spmd(nc, in_maps, core_ids=list(range(NC)),
                               trace=trace)
    out = np.empty((B, S, V), np.float32)
    for b in range(B):
        ltab = res.results[b * G]["ltab"]             # [128, 512]
        tab = ltab.reshape(128, 2, 256).transpose(1, 0, 2).reshape(256, 256)
        out[b] = tab.T[byte_seq[b]]                   # [S, 256]
    return out, res


def kernel(**inputs) -> np.ndarray:
    out, _ = run_device(inputs, trace=False)
    return out


# revision 23
# speedup vs baseline: 1.0422x; 1.0422x over previous
"""BLT model TRN2 kernel — nn_BLTModel_13872744366807.

Strategy v3:
- Vocab collapse (v1): byte-axis path depends only on byte VALUE, so the
  [B,4096,*] byte axis collapses to a [B,256,*] vocab table; pooling is a
  host histogram matrix x emb; final output is a host gather.
- DP-2 x TP-4: cores 0-3 batch 0, cores 4-7 batch 1 (256 patch tokens).
- Attention reduction via AllGather of head-sharded attn outputs (bf16,
  ~9.5us vs ~18.5us AllReduce), Wo applied replicated per core.
- MLP reduction via 2 token-chunked fp16 AllReduces (~13us each),
  pipelined against the other chunk's w1/gelu/w2 and the next layer's
  qkv/scores lead-in (causal chunking: queries 0-127 need keys 0-127 only).
- LayerNorm: gpsimd partition_all_reduce produces partition-broadcast
  stats (no M=1 reduce / K=1 broadcast matmuls on PE); direct
  normalization z=(h-mu)*rsig with LN gains host-folded into weights;
  rsqrt = exp(-0.5*ln(var+eps)) so attention Exp and LN share one act
  table; gelu = Gelu_apprx_tanh (2e-4 end-to-end); dummy activations
  prefetch table swaps off the critical path.
- V projections computed in token-major layout (z as lhsT), so AV
  matmuls need no PE transposes; v/k/q biases host-folded where exact.
"""
import numpy as np
import ml_dtypes
import concourse.bacc as bacc
import concourse.bass as bass
import concourse.bass_isa as bass_isa
import concourse.mybir as mybir
from concourse import tile
from concourse.bass_utils import run_bass_kernel_spmd
from concourse.bass_interp import get_hw_module

F32 = mybir.dt.float32
BF16 = mybir.dt.bfloat16
FP16 = mybir.dt.float16
AF = mybir.ActivationFunctionType
ALU = mybir.AluOpType
ROP = bass_isa.ReduceOp
BF = ml_dtypes.bfloat16

L, B, S, P, H, V, NC = 4, 2, 4096, 256, 1024, 256, 8
G = 4                  # tensor-parallel group size
EPS = 1e-6
RG = [[0, 1, 2, 3], [4, 5, 6, 7]]

_CACHE = {}


def _trace(skip_kvn_ln):
    nc = bacc.Bacc("TRN2", target_bir_lowering=False, debug=False,
                   num_devices=NC)
    d = {}

    def inp(name, shape, dt=BF16):
        d[name] = nc.dram_tensor(name, shape, dt, kind="ExternalInput").ap()

    inp("wqkv", [L, 128, 4096])          # m-tiles [q0,k0,q1,k1] per kc
    inp("bqkv4", [L, 128, 4], F32)
    inp("wv", [L, 128, 2048])            # rhs layout [128, kc, 256]
    inp("wo", [L, 128, 8192])            # FULL Wo [128, kc, 1024]
    inp("bo8", [L, 128, 8], F32)
    inp("w1", [L, 128, 8192])
    inp("b1c", [L, 128, 8], F32)
    inp("w2", [L, 128, 8192])
    inp("b28", [L, 128, 8], F32)
    inp("wq", [128, 2048]); inp("wk", [128, 2048])
    inp("wvca", [128, 2048])             # rhs layout [128, kc, 256]
    inp("bq", [128, 2], F32); inp("bk", [128, 2], F32)
    inp("cawoT", [128, 2048])
    inp("headw", [128, 2048])
    inp("headb", [128, 2], F32)
    inp("embT", [128, 2048])
    inp("embS", [128, 2048])
    inp("cnt", [128, 512])
    inp("tri", [128, 128])
    inp("fng", [128, 8], F32); inp("fnb", [128, 8], F32)
    inp("cag", [128, 8], F32); inp("cab", [128, 8], F32)
    out_d = nc.dram_tensor("ltab", [128, 512], F32, kind="ExternalOutput").ap()

    with tile.TileContext(nc) as tc:
        with (
            tc.tile_pool(name="const", bufs=1) as cp,
            tc.tile_pool(name="sb", bufs=1) as sbp,
            tc.tile_pool(name="wts", bufs=1) as wp,
            tc.tile_pool(name="wts1", bufs=1) as wps,
            tc.tile_pool(name="tmp", bufs=2) as tp,
            tc.tile_pool(name="tps", bufs=2) as tps,
            tc.tile_pool(name="pp", bufs=2, space="PSUM") as pp,
            tc.tile_pool(name="pc", bufs=3, space="PSUM") as pc,
            tc.tile_pool(name="pa", bufs=3, space="PSUM") as pa,
            tc.tile_pool(name="dram", bufs=1, space="DRAM") as dp,
        ):
            # cc warm-up: tiny AllReduce to absorb launch skew
            wbin = dp.tile([128, 2], F32, tag="wrmi")
            wbout = dp.tile([128, 2], F32, tag="wrmo")
            nc.sync.dma_start(wbin[:], d["headb"][:])
            nc.gpsimd.collective_compute(
                "AllReduce", ALU.add, replica_groups=RG,
                ins=[wbin[:].opt()], outs=[wbout[:].opt()])

            # ---------------- constants ----------------
            def cload(name, shape, dt=BF16):
                t_ = cp.tile(shape, dt, tag=name)
                nc.sync.dma_start(t_[:], d[name][:])
                return t_

            tri_t = cload("tri", [128, 128])
            fng_t = cload("fng", [128, 8], F32); fnb_t = cload("fnb", [128, 8], F32)
            cag_t = cload("cag", [128, 8], F32); cab_t = cload("cab", [128, 8], F32)
            headb_t = cload("headb", [128, 2], F32)
            bq_t = cload("bq", [128, 2], F32); bk_t = cload("bk", [128, 2], F32)
            embS_t = cp.tile([128, 2, 1024], BF16, tag="embS")
            for q in range(2):
                nc.scalar.dma_start(embS_t[:, q, :], d["embS"][:].rearrange(
                    "p (vc x) -> p vc x", vc=2)[:, q, :])
            cnt_t = cp.tile([128, 2, 256], BF16, tag="cnt")
            nc.scalar.dma_start(cnt_t[:], d["cnt"][:].rearrange(
                "p (vc x) -> p vc x", vc=2))

            # ---------------- persistent activations ----------------
            h_t = sbp.tile([128, 8, 256], F32, tag="h")
            h16_t = sbp.tile([128, 8, 256], BF16, tag="h16")
            z_t = sbp.tile([128, 8, 256], BF16, tag="z")
            sq_t = sbp.tile([128, 8, 256], BF16, tag="sq")
            qk_t = sbp.tile([128, 2, 2, 256], BF16, tag="qk")
            qkh2_t = sbp.tile([64, 2, 2, 256], BF16, tag="qkh2")
            vtok_t = sbp.tile([128, 2, 256], BF16, tag="vtok")
            A_t = sbp.tile([128, 2, 256], BF16, tag="A")
            Af_t = sbp.tile([128, 8, 256], BF16, tag="Af")
            mo_t = sbp.tile([128, 8, 256], FP16, tag="mo")
            ari_t = sbp.tile([128, 8, 256], FP16, tag="ari")
            gu_t = sbp.tile([128, 8, 256], BF16, tag="gu")
            qn_t = gu_t  # tail_pre finishes with qn before gelu writes gu
            scrap_t = sbp.tile([1, 8], F32, tag="scrap")
            eps_t = sbp.tile([128, 1], F32, tag="eps")
            nc.vector.memset(eps_t[:], EPS)

            def prefetch_table(func):
                nc.scalar.activation(scrap_t[0:1, 0:1], tri_t[0:1, 0:1], func)

            # ---------------- LN stats + normalize helper ----------------
            def stats_z(src16, c, out_z, w, gain=None, bias=None):
                """src16: [128, 8, 256] bf16. Token slice cs=[c*w,(c+1)*w).
                Writes out_z[:, :, cs] = (x-mu)*rsig (*gain+bias per kc)."""
                cs = slice(c * w, c * w + w)
                nc.vector.tensor_tensor(out=sq_t[:, :, cs], in0=src16[:, :, cs],
                                        in1=src16[:, :, cs], op=ALU.mult)
                t4 = tp.tile([128, 4, 256], BF16, tag="tr4")
                t2 = tp.tile([128, 2, 256], BF16, tag="tr2")
                t1 = tps.tile([128, 512], BF16, tag="t1p")
                for (src_, dst_col) in ((src16, 0), (sq_t, 1)):
                    nc.vector.tensor_tensor(
                        out=t4[:, :, cs], in0=src_[:, 0:4, cs],
                        in1=src_[:, 4:8, cs], op=ALU.add)
                    nc.vector.tensor_tensor(
                        out=t2[:, :, :w], in0=t4[:, 0:2, cs],
                        in1=t4[:, 2:4, cs], op=ALU.add)
                    nc.vector.tensor_tensor(
                        out=t1[:, dst_col * w:(dst_col + 1) * w],
                        in0=t2[:, 0, :w], in1=t2[:, 1, :w], op=ALU.add)
                sb = tps.tile([128, 512], F32, tag="sb")
                nc.gpsimd.partition_all_reduce(
                    sb[:, 0:2 * w], t1[:, 0:2 * w],
                    channels=128, reduce_op=ROP.add)
                inv = 1.0 / float(H)
                mu = tps.tile([128, 256], F32, tag="mu")
                nc.vector.tensor_scalar_mul(mu[:, :w], sb[:, 0:w], inv)
                var = tps.tile([128, 256], F32, tag="var")
                # var = sumsq/H - mu*mu  (+EPS folded into Ln bias)
                nc.vector.scalar_tensor_tensor(
                    out=var[:, :w], in0=mu[:, :w], scalar=-1.0, in1=mu[:, :w],
                    op0=ALU.mult, op1=ALU.mult)
                nc.vector.scalar_tensor_tensor(
                    out=var[:, :w], in0=sb[:, w:2 * w], scalar=inv,
                    in1=var[:, :w], op0=ALU.mult, op1=ALU.add)
                vln = tps.tile([128, 256], F32, tag="vln")
                nc.scalar.activation(vln[:, :w], var[:, :w], AF.Ln,
                                     bias=eps_t[:, 0:1])
                rsig = tps.tile([128, 256], F32, tag="rsig")
                nc.scalar.activation(rsig[:, :w], vln[:, :w], AF.Exp, scale=-0.5)
                r16 = tps.tile([128, 256], BF16, tag="r16")
                nc.vector.tensor_copy(r16[:, :w], rsig[:, :w])
                ms16 = tps.tile([128, 256], BF16, tag="ms16")
                nc.vector.tensor_tensor(out=ms16[:, :w], in0=mu[:, :w],
                                        in1=r16[:, :w], op=ALU.mult)
                for kc in range(8):
                    nc.vector.tensor_tensor(out=out_z[:, kc, cs],
                                            in0=src16[:, kc, cs],
                                            in1=r16[:, :w], op=ALU.mult)
                    nc.vector.tensor_tensor(out=out_z[:, kc, cs],
                                            in0=out_z[:, kc, cs],
                                            in1=ms16[:, :w], op=ALU.subtract)
                    if gain is not None:
                        nc.vector.tensor_scalar(out=out_z[:, kc, cs],
                                                in0=out_z[:, kc, cs],
                                                scalar1=gain[:, kc:kc + 1],
                                                scalar2=bias[:, kc:kc + 1],
                                                op0=ALU.mult, op1=ALU.add)

            # ---------------- batch-independent tail precompute -------------
            embT_t = sbp.tile([128, 8, 256], BF16, tag="embT")
            headw_t = sbp.tile([128, 8, 256], BF16, tag="headw")
            wqca_t = sbp.tile([128, 8, 256], BF16, tag="wqca")
            cawoT_t = sbp.tile([128, 8, 256], BF16, tag="cawoT")

            def tail_loads():
                for (t_, nm) in ((embT_t, "embT"), (headw_t, "headw"),
                                 (wqca_t, "wq"), (cawoT_t, "cawoT")):
                    nc.scalar.dma_start(t_[:], d[nm][:].rearrange(
                        "p (kc x) -> p kc x", kc=8))

            qT_t = sbp.tile([128, 2, 256], BF16, tag="qT")
            w2c_t = sbp.tile([128, 2, 256], BF16, tag="w2c")
            et_t = sbp.tile([128, 2, 256], F32, tag="et")

            def tail_pre():
                if skip_kvn_ln:
                    stats_z(embT_t, 0, qn_t, 256)
                else:
                    stats_z(embT_t, 0, qn_t, 256, gain=cag_t, bias=cab_t)
                for h2 in range(2):
                    ps = pp.tile([128, 256], F32, tag="mm")
                    for kc in range(8):
                        nc.tensor.matmul(ps[:],
                                         wqca_t[:, kc, h2 * 128:(h2 + 1) * 128],
                                         qn_t[:, kc, :],
                                         start=(kc == 0), stop=(kc == 7))
                    nc.vector.tensor_scalar(out=qT_t[:, h2, :], in0=ps[:],
                                            scalar1=bq_t[:, h2:h2 + 1],
                                            scalar2=None, op0=ALU.add)
                for lt in range(2):
                    ps_e = pp.tile([128, 256], F32, tag="mm")
                    for kc in range(8):
                        nc.tensor.matmul(
                            ps_e[:], headw_t[:, kc, lt * 128:(lt + 1) * 128],
                            embT_t[:, kc, :],
                            start=(kc == 0), stop=(kc == 7))
                    nc.vector.tensor_copy(et_t[:, lt, :], ps_e[:])
                for h2 in range(2):
                    ps = pp.tile([128, 256], F32, tag="mm")
                    for kc in range(8):
                        nc.tensor.matmul(
                            ps[:], cawoT_t[:, kc, h2 * 128:(h2 + 1) * 128],
                            headw_t[:, kc, :],
                            start=(kc == 0), stop=(kc == 7))
                    nc.vector.tensor_copy(w2c_t[:, h2, :], ps[:])

            # ---------------- patch pooling: h = patchesT ----------------
            for ti in range(8):
                ps = pp.tile([128, 256], F32, tag="mm")
                for vc in range(2):
                    nc.tensor.matmul(ps[:],
                                     embS_t[:, vc, ti * 128:(ti + 1) * 128],
                                     cnt_t[:, vc, :],
                                     start=(vc == 0), stop=(vc == 1))
                nc.vector.tensor_copy(h_t[:, ti, :], ps[:])
                nc.scalar.activation(h16_t[:, ti, :], ps[:], AF.Copy)

            # initial ln1(L0) for both chunks
            stats_z(h16_t, 0, z_t, 128)
            stats_z(h16_t, 1, z_t, 128)

            # ---------------- transformer layers ----------------
            wqkv_t = wp.tile([128, 8, 512], BF16, tag="wqkv")
            wv_t = wp.tile([128, 8, 256], BF16, tag="wv")
            bqkv4_t = wp.tile([128, 4], F32, tag="bqkv4")
            wo_t = wps.tile([128, 8, 1024], BF16, tag="wo")
            bo8_t = wp.tile([128, 8], F32, tag="bo8")
            w1_t = wps.tile([128, 8, 1024], BF16, tag="w1")
            w2_t = wps.tile([128, 8, 1024], BF16, tag="w2")
            b1c_t = wp.tile([128, 8], F32, tag="b1c")
            b28_t = wp.tile([128, 8], F32, tag="b28")

            def load_attn_w(l):
                for q in range(2):
                    nc.scalar.dma_start(
                        wqkv_t[:, q * 4:(q + 1) * 4, :],
                        d["wqkv"][l].rearrange("p (kc x) -> p kc x", kc=8)
                        [:, q * 4:(q + 1) * 4, :])
                nc.scalar.dma_start(wv_t[:], d["wv"][l].rearrange(
                    "p (kc x) -> p kc x", kc=8))
                nc.scalar.dma_start(bqkv4_t[:], d["bqkv4"][l])

            def load_mlp_w(l):
                for (t_, nm, spl) in ((wo_t, "wo", 4), (w1_t, "w1", 4),
                                      (w2_t, "w2", 4)):
                    for q in range(spl):
                        nc.scalar.dma_start(
                            t_[:, q * 2:(q + 1) * 2, :],
                            d[nm][l].rearrange("p (kc x) -> p kc x", kc=8)
                            [:, q * 2:(q + 1) * 2, :])
                nc.scalar.dma_start(bo8_t[:], d["bo8"][l])
                nc.scalar.dma_start(b1c_t[:], d["b1c"][l])
                nc.scalar.dma_start(b28_t[:], d["b28"][l])

            for l in range(4):
                load_attn_w(l)
                load_mlp_w(l)
                if l == 0:
                    tail_loads()

                # ---- qkv projections (both chunks) + v_tok ----
                for c in range(2):
                    cs = slice(c * 128, c * 128 + 128)
                    for m in range(4):
                        p_, j = m // 2, m % 2
                        ps_f = pc.tile([128, 256], F32, tag="mm")
                        ps = ps_f[:, 0:128]
                        for kc in range(8):
                            nc.tensor.matmul(
                                ps[:], wqkv_t[:, kc, m * 128:(m + 1) * 128],
                                z_t[:, kc, cs],
                                start=(kc == 0), stop=(kc == 7))
                        nc.vector.tensor_scalar(
                            out=qk_t[:, p_, j, cs], in0=ps[:],
                            scalar1=bqkv4_t[:, m:m + 1],
                            scalar2=None, op0=ALU.add)
                        nc.sync.dma_start(qkh2_t[:, p_, j, cs],
                                          qk_t[64:128, p_, j, cs])
                    ps_v = pc.tile([128, 256], F32, tag="mm")
                    for kc in range(8):
                        nc.tensor.matmul(ps_v[:], z_t[:, kc, cs],
                                         wv_t[:, kc, :],
                                         start=(kc == 0), stop=(kc == 7))
                    nc.vector.tensor_copy(vtok_t[:, c, :], ps_v[:])

                # ---- attention: chunk c queries attend key-blocks <= c ----
                ag_outs = []
                em_c0 = tp.tile([128, 4, 128], BF16, tag="emc0")
                em_c1 = tp.tile([128, 2, 4, 128], BF16, tag="emc1")
                ems_c1 = tp.tile([128, 4, 128], BF16, tag="emsc1")
                for c in range(2):
                    cs = slice(c * 128, c * 128 + 128)
                    for hh in range(2):
                        for p_ in range(2):
                            hidx = p_ * 2 + hh
                            src = qk_t if hh == 0 else qkh2_t
                            qT = src[0:64, p_, 0, cs]
                            for kt in range(c + 1):
                                ks = slice(kt * 128, kt * 128 + 128)
                                kT = src[0:64, p_, 1, ks]
                                ps_sf = pa.tile([128, 256], F32, tag="att")
                                ps_s = ps_sf[:, 0:128]
                                nc.tensor.matmul(ps_s[:], kT, qT,
                                                 start=True, stop=True)
                                if c == kt:
                                    ex = tp.tile([128, 128], BF16, tag="ex")
                                    nc.scalar.activation(ex[:], ps_s[:],
                                                         AF.Exp, scale=0.125)
                                    dst = (em_c0[:, hidx, :] if c == 0 else
                                           em_c1[:, kt, hidx, :])
                                    nc.vector.tensor_tensor(
                                        out=dst, in0=ex[:], in1=tri_t[:],
                                        op=ALU.mult)
                                else:
                                    nc.scalar.activation(
                                        em_c1[:, kt, hidx, :], ps_s[:],
                                        AF.Exp, scale=0.125)
                    # denominators (partition reduce, output broadcast)
                    den = tps.tile([128, 4, 128], F32, tag="den")
                    if c == 0:
                        nc.gpsimd.partition_all_reduce(
                            den[:].rearrange("p a b -> p (a b)"),
                            em_c0[:].rearrange("p a b -> p (a b)"),
                            channels=128, reduce_op=ROP.add)
                    else:
                        nc.vector.tensor_tensor(out=ems_c1[:],
                                                in0=em_c1[:, 0], in1=em_c1[:, 1],
                                                op=ALU.add)
                        nc.gpsimd.partition_all_reduce(
                            den[:].rearrange("p a b -> p (a b)"),
                            ems_c1[:].rearrange("p a b -> p (a b)"),
                            channels=128, reduce_op=ROP.add)
                    rec = tps.tile([128, 4, 128], F32, tag="rec")
                    nc.vector.reciprocal_approx_fast(
                        out=rec[:].rearrange("p a b -> p (a b)"),
                        in_=den[:].rearrange("p a b -> p (a b)"))
                    # AV + scale
                    for p_ in range(2):
                        ps_of = pa.tile([128, 256], F32, tag="att")
                        ps_o = ps_of[:, 0:128]
                        for hh in range(2):
                            hidx = p_ * 2 + hh
                            for kt in range(c + 1):
                                ks = slice(kt * 128, kt * 128 + 128)
                                em_ap = (em_c0[:, hidx, :] if c == 0 else
                                         em_c1[:, kt, hidx, :])
                                nc.tensor.matmul(
                                    ps_o[hh * 64:(hh + 1) * 64, :],
                                    vtok_t[:, kt, hidx * 64:(hidx + 1) * 64],
                                    em_ap, start=(kt == 0), stop=(kt == c))
                        for hh in range(2):
                            hidx = p_ * 2 + hh
                            nc.vector.tensor_tensor(
                                out=A_t[hh * 64:(hh + 1) * 64, p_, cs],
                                in0=ps_o[hh * 64:(hh + 1) * 64, :],
                                in1=rec[hh * 64:(hh + 1) * 64, hidx, :],
                                op=ALU.mult)
                    # stage + AllGather this chunk's attn output
                    ag_in = dp.tile([128, 2, 128], BF16, tag=f"agi{l}{c}")
                    ag_out = dp.tile([4, 128, 2, 128], BF16, tag=f"ago{l}{c}")
                    nc.sync.dma_start(ag_in[:], A_t[:, :, cs])
                    nc.gpsimd.collective_compute(
                        "AllGather", ALU.bypass, replica_groups=RG,
                        ins=[ag_in[:].opt()], outs=[ag_out[:].opt()])
                    ag_outs.append(ag_out)
                    if l == 0 and c == 0:
                        tail_pre()
                # fetches after both stages: sync queue stays monotone
                for c in range(2):
                    cs = slice(c * 128, c * 128 + 128)
                    for r_ in range(4):
                        nc.sync.dma_start(Af_t[:, 2 * r_:2 * r_ + 2, cs],
                                          ag_outs[c][r_])

                # ---- wo (full, replicated) + resid + ln2 + mlp, per chunk --
                arb = [None, None]
                for c in range(2):
                    cs = slice(c * 128, c * 128 + 128)
                    for m in range(8):
                        ps_f = pc.tile([128, 256], F32, tag="mm")
                        ps = ps_f[:, 0:128]
                        for kc in range(8):
                            nc.tensor.matmul(
                                ps[:], wo_t[:, kc, m * 128:(m + 1) * 128],
                                Af_t[:, kc, cs],
                                start=(kc == 0), stop=(kc == 7))
                        nc.vector.scalar_tensor_tensor(
                            out=h_t[:, m, cs], in0=ps[:],
                            scalar=bo8_t[:, m:m + 1], in1=h_t[:, m, cs],
                            op0=ALU.add, op1=ALU.add)
                        nc.scalar.activation(h16_t[:, m, cs], h_t[:, m, cs],
                                             AF.Copy)
                    stats_z(h16_t, c, z_t, 128)
                    prefetch_table(AF.Gelu_apprx_tanh)
                    for m in range(8):
                        ps_f = pc.tile([128, 256], F32, tag="mm")
                        ps = ps_f[:, 0:128]
                        for kc in range(8):
                            nc.tensor.matmul(
                                ps[:], w1_t[:, kc, m * 128:(m + 1) * 128],
                                z_t[:, kc, cs],
                                start=(kc == 0), stop=(kc == 7))
                        nc.scalar.activation(gu_t[:, m, cs], ps[:],
                                             AF.Gelu_apprx_tanh,
                                             bias=b1c_t[:, m:m + 1])
                    for m in range(8):
                        ps_f = pc.tile([128, 256], F32, tag="mm")
                        ps = ps_f[:, 0:128]
                        for kc in range(8):
                            nc.tensor.matmul(
                                ps[:], w2_t[:, kc, m * 128:(m + 1) * 128],
                                gu_t[:, kc, cs],
                                start=(kc == 0), stop=(kc == 7))
                        nc.vector.tensor_scalar(out=mo_t[:, m, cs], in0=ps[:],
                                                scalar1=b28_t[:, m:m + 1],
                                                scalar2=None, op0=ALU.add)
                    prefetch_table(AF.Exp)
                    ar_in = dp.tile([128, 1024], FP16, tag=f"arin{l}{c}")
                    ar_out = dp.tile([128, 1024], FP16, tag=f"arou{l}{c}")
                    nc.sync.dma_start(
                        ar_in[:].rearrange("p (a b) -> p a b", a=8),
                        mo_t[:, :, cs])
                    nc.gpsimd.collective_compute(
                        "AllReduce", ALU.add, replica_groups=RG,
                        ins=[ar_in[:].opt()], outs=[ar_out[:].opt()])
                    arb[c] = ar_out

                # ---- mlp resid + next ln1 (or final norm), per chunk ----
                for c in range(2):
                    cs = slice(c * 128, c * 128 + 128)
                    nc.sync.dma_start(
                        ari_t[:, :, cs],
                        arb[c][:].rearrange("p (a b) -> p a b", a=8))
                    for m in range(8):
                        nc.vector.tensor_tensor(out=h_t[:, m, cs],
                                                in0=h_t[:, m, cs],
                                                in1=ari_t[:, m, cs], op=ALU.add)
                        nc.scalar.activation(h16_t[:, m, cs], h_t[:, m, cs],
                                             AF.Copy)
                    if l < 3 or skip_kvn_ln:
                        stats_z(h16_t, c, z_t, 128)
                    else:
                        stats_z(h16_t, c, z_t, 128, gain=fng_t, bias=fnb_t)

            # ---------------- tail: final norm / CA / logits ----------------
            # z_t now holds pf = ln(h)*fng+fnb (or plain z if skip).
            if not skip_kvn_ln:
                # kvn = ln(pf)*cag+cab ; pf currently in z_t -> copy to h16
                for ti in range(8):
                    nc.vector.tensor_copy(h16_t[:, ti, :], z_t[:, ti, :])
                stats_z(h16_t, 0, z_t, 256, gain=cag_t, bias=cab_t)
            kvn_t = z_t

            wkca_t = cp.tile([128, 8, 256], BF16, tag="wkca")
            nc.scalar.dma_start(wkca_t[:], d["wk"][:].rearrange(
                "p (kc x) -> p kc x", kc=8))
            wvca_t = cp.tile([128, 8, 256], BF16, tag="wvca")
            nc.scalar.dma_start(wvca_t[:], d["wvca"][:].rearrange(
                "p (kc x) -> p kc x", kc=8))

            kT_t = sbp.tile([128, 2, 256], BF16, tag="kT")
            vtca_t = sbp.tile([128, 2, 256], BF16, tag="vtca")
            for h2 in range(2):
                ps = pp.tile([128, 256], F32, tag="mm")
                for kc in range(8):
                    nc.tensor.matmul(ps[:],
                                     wkca_t[:, kc, h2 * 128:(h2 + 1) * 128],
                                     kvn_t[:, kc, :],
                                     start=(kc == 0), stop=(kc == 7))
                nc.vector.tensor_scalar(out=kT_t[:, h2, :], in0=ps[:],
                                        scalar1=bk_t[:, h2:h2 + 1],
                                        scalar2=None, op0=ALU.add)
            for tb in range(2):
                ps_v = pp.tile([128, 256], F32, tag="mm")
                for kc in range(8):
                    nc.tensor.matmul(ps_v[:],
                                     kvn_t[:, kc, tb * 128:(tb + 1) * 128],
                                     wvca_t[:, kc, :],
                                     start=(kc == 0), stop=(kc == 7))
                nc.vector.tensor_copy(vtca_t[:, tb, :], ps_v[:])

            # ---------------- CA attention (2 heads, dh=128) ----------------
            O_t = sbp.tile([128, 2, 256], BF16, tag="O")
            emca = tp.tile([128, 2, 2, 256], BF16, tag="emca")
            for h2 in range(2):
                for kt in range(2):
                    ps_s = pa.tile([128, 256], F32, tag="att")
                    nc.tensor.matmul(
                        ps_s[:], kT_t[:, h2, kt * 128:(kt + 1) * 128],
                        qT_t[:, h2, :], start=True, stop=True)
                    nc.scalar.activation(emca[:, h2, kt, :], ps_s[:], AF.Exp,
                                         scale=float(1.0 / np.sqrt(128.0)))
            emsca = tp.tile([128, 2, 256], BF16, tag="emsca")
            nc.vector.tensor_tensor(out=emsca[:], in0=emca[:, :, 0, :],
                                    in1=emca[:, :, 1, :], op=ALU.add)
            denca = tps.tile([128, 2, 256], F32, tag="denca")
            nc.gpsimd.partition_all_reduce(
                denca[:].rearrange("p a b -> p (a b)"),
                emsca[:].rearrange("p a b -> p (a b)"),
                channels=128, reduce_op=ROP.add)
            recca = tps.tile([128, 2, 256], F32, tag="recca")
            nc.vector.reciprocal_approx_fast(
                out=recca[:].rearrange("p a b -> p (a b)"),
                in_=denca[:].rearrange("p a b -> p (a b)"))
            for h2 in range(2):
                ps_o = pa.tile([128, 256], F32, tag="att")
                for kt in range(2):
                    nc.tensor.matmul(
                        ps_o[:], vtca_t[:, kt, h2 * 128:(h2 + 1) * 128],
                        emca[:, h2, kt, :], start=(kt == 0), stop=(kt == 1))
                nc.vector.tensor_tensor(out=O_t[:, h2, :], in0=ps_o[:],
                                        in1=recca[:, h2, :], op=ALU.mult)

            # ---------------- logits partials + AR ----------------
            lp_t = sbp.tile([128, 2, 256], FP16, tag="lp")
            for lt in range(2):
                ps = pp.tile([128, 256], F32, tag="mm")
                for h2 in range(2):
                    nc.tensor.matmul(ps[:],
                                     w2c_t[:, h2, lt * 128:(lt + 1) * 128],
                                     O_t[:, h2, :],
                                     start=(h2 == 0), stop=(h2 == 1))
                nc.vector.tensor_copy(lp_t[:, lt, :], ps[:])
            lbin = dp.tile([128, 512], FP16, tag="lci")
            lbout = dp.tile([128, 512], FP16, tag="lco")
            nc.sync.dma_start(lbin[:], lp_t[:])
            nc.gpsimd.collective_compute(
                "AllReduce", ALU.add, replica_groups=RG,
                ins=[lbin[:].opt()], outs=[lbout[:].opt()])
            lar_t = sbp.tile([128, 2, 256], FP16, tag="lar")
            nc.sync.dma_start(lar_t[:], lbout[:])

            out_t = sbp.tile([128, 2, 256], F32, tag="outt")
            for lt in range(2):
                tb = tp.tile([128, 256], F32, tag="tb")
                nc.vector.tensor_scalar(out=tb[:], in0=lar_t[:, lt, :],
                                        scalar1=headb_t[:, lt:lt + 1],
                                        scalar2=None, op0=ALU.add)
                nc.vector.tensor_tensor(out=out_t[:, lt, :],
                                        in0=tb[:], in1=et_t[:, lt, :],
                                        op=ALU.add)
            nc.sync.dma_start(out_d[:], out_t[:])

    nc.compile()
    nc.m = get_hw_module(nc.m)
    return nc


# --------------------------------------------------------------------------
# host side
# --------------------------------------------------------------------------
def _shuf(M):
    """[K, X] -> [128, (K//128)*X] laid out as [p, kc, x]."""
    K, X = M.shape
    return np.ascontiguousarray(
        M.reshape(K // 128, 128, X).transpose(1, 0, 2).reshape(128, -1))


def _bf(M):
    return np.ascontiguousarray(M).astype(BF)


def _prep(inputs):
    f = lambda k: np.asarray(inputs[k], np.float32)
    byte_seq = np.asarray(inputs["byte_seq"])
    bd = np.asarray(inputs["patch_boundaries"])
    emb = f("emb")

    # patch histogram matrix
    pos = np.arange(S)
    pid = np.stack([np.searchsorted(bd[b], pos, side="right") for b in range(B)])
    pid = np.clip(pid, 0, P - 1)
    Cn = np.zeros((B, P, V), np.float32)
    for b in range(B):
        np.add.at(Cn[b], (pid[b], byte_seq[b]), 1.0)
    cnts = Cn.sum(-1)
    Cn /= np.maximum(cnts, 1.0)[..., None]

    g1, b1a = f("g_ln1_g"), f("g_ln1_b")
    g2, b2a = f("g_ln2_g"), f("g_ln2_b")
    Wqkv, bqkv = f("g_wqkv"), f("g_bqkv")
    Wo, bo = f("g_wo"), f("g_bo")
    W1, b1 = f("g_w1"), f("g_b1")
    W2, b2 = f("g_w2"), f("g_b2")

    Wq_f = g1[:, :, None] * Wqkv                       # [L, H, 3H]
    biasq = np.einsum("lh,lho->lo", b1a, Wqkv) + bqkv  # [L, 3H]
    W1_f = g2[:, :, None] * W1
    bias1 = np.einsum("lh,lho->lo", b2a, W1) + b1

    # v bias folds into wo bias: attn_out = A_nobias + bv  (softmax sums to 1)
    bv_full = biasq[:, 2 * H:]                          # [L, H]
    bo_full = bo + np.einsum("lh,lho->lo", bv_full, Wo)  # [L, H]

    ca_wqkv, ca_bqkv = f("ca_wqkv"), f("ca_bqkv")
    ca_wo, ca_bo = f("ca_wo"), f("ca_bo")
    head_w, head_b = f("head_w"), f("head_b")
    headb_full = (head_b + ca_bo @ head_w
                  + (ca_bqkv[2 * H:] @ ca_wo) @ head_w)  # [256]

    tri = (np.arange(128)[:, None] <= np.arange(128)[None, :]).astype(
        np.float32)

    shared = {
        "headw": _bf(_shuf(head_w)),
        "headb": np.ascontiguousarray(headb_full.reshape(2, 128).T),
        "embT": _bf(_shuf(np.ascontiguousarray(emb.T))),
        "embS": _bf(_shuf(emb)),
        "tri": _bf(tri),
        "fng": np.ascontiguousarray(f("fn_g").reshape(8, 128).T),
        "fnb": np.ascontiguousarray(f("fn_b").reshape(8, 128).T),
        "cag": np.ascontiguousarray(f("ca_ln_g").reshape(8, 128).T),
        "cab": np.ascontiguousarray(f("ca_ln_b").reshape(8, 128).T),
        "wo": _bf(np.stack([_shuf(Wo[l]) for l in range(L)])),
        "bo8": np.ascontiguousarray(
            bo_full.reshape(L, 8, 128).transpose(0, 2, 1)),
    }

    in_maps = []
    for c in range(NC):
        g, r = c // G, c % G
        m = dict(shared)
        m["cnt"] = _bf(_shuf(np.ascontiguousarray(Cn[g].T)))

        # q/k m-tiles: [q_p0, k_p0, q_p1, k_p1]
        qk_cols = np.concatenate([
            j * H + 256 * r + 128 * p + np.arange(128)
            for p in range(2) for j in range(2)])
        m["wqkv"] = np.stack([_bf(_shuf(Wq_f[l][:, qk_cols]))
                              for l in range(L)])
        m["bqkv4"] = np.ascontiguousarray(
            biasq[:, qk_cols].reshape(L, 4, 128).transpose(0, 2, 1))
        # v as rhs [128, kc, 256]: cols 2H + 256r .. +256
        vsl = slice(2 * H + 256 * r, 2 * H + 256 * (r + 1))
        m["wv"] = np.stack([_bf(_shuf(Wq_f[l][:, vsl])) for l in range(L)])

        csl = slice(1024 * r, 1024 * (r + 1))
        m["w1"] = np.stack([_bf(_shuf(W1_f[l][:, csl])) for l in range(L)])
        m["b1c"] = np.ascontiguousarray(
            bias1[:, csl].reshape(L, 8, 128).transpose(0, 2, 1))
        m["w2"] = np.stack([_bf(_shuf(W2[l][csl, :])) for l in range(L)])
        m["b28"] = np.ascontiguousarray(
            b2.reshape(L, 8, 128).transpose(0, 2, 1) / G)

        hsl = slice(256 * r, 256 * (r + 1))
        m["wq"] = _bf(_shuf(ca_wqkv[:, hsl]))
        m["wk"] = _bf(_shuf(ca_wqkv[:, np.arange(256) + H + 256 * r]))
        m["wvca"] = _bf(_shuf(ca_wqkv[:, np.arange(256) + 2 * H + 256 * r]))
        m["bq"] = np.ascontiguousarray(ca_bqkv[hsl].reshape(2, 128).T)
        m["bk"] = np.ascontiguousarray(
            ca_bqkv[H + 256 * r:H + 256 * (r + 1)].reshape(2, 128).T)
        m["cawoT"] = _bf(np.concatenate([
            _shuf(np.ascontiguousarray(
                ca_wo[256 * r + 128 * h2:256 * r + 128 * (h2 + 1), :].T))
            .reshape(128, 8, 128) for h2 in range(2)], axis=2)
            .reshape(128, -1))
        in_maps.append(m)
    return in_maps, byte_seq


def run_device(inputs, trace=False):
    skip = (np.allclose(np.asarray(inputs["fn_g"]), 1.0)
            and np.allclose(np.asarray(inputs["fn_b"]), 0.0)
            and np.allclose(np.asarray(inputs["ca_ln_g"]), 1.0)
            and np.allclose(np.asarray(inputs["ca_ln_b"]), 0.0))
    key = ("nc", skip)
    if key not in _CACHE:
        _CACHE[key] = _trace(skip)
    nc = _CACHE[key]
    in_maps, byte_seq = _prep(inputs)
    res = run_bass_kernel_spmd(nc, in_maps, core_ids=list(range(NC)),
                               trace=trace)
    out = np.empty((B, S, V), np.float32)
    for b in range(B):
        ltab = res.results[b * G]["ltab"]             # [128, 512]
        tab = ltab.reshape(128, 2, 256).transpose(1, 0, 2).reshape(256, 256)
        out[b] = tab.T[byte_seq[b]]                   # [S, 256]
    return out, res


def kernel(**inputs) -> np.ndarray:
    out, _ = run_device(inputs, trace=False)
    return out
